# revision 17
# baseline (speedup 1.0000x reference)
"""Trainium2 Bass kernel for the SRNN adapter problem.

Strategy (8 cores, data-parallel over batch B=256 -> 32 per core). Measured
cost model (differential nrep timing on this box): execution is stream-bound
on the PE -- an fp32 matmul streams its moving operand at 4 cycles/row
@2.4GHz, bf16 at 1 cycle/row, with ~0.1-0.3us per-MM overhead; For_i trips
cost their executed work (the per-static-instruction model of the earlier
session was a mismeasurement).

The recurrent matmul z @ W_eff^T dominates. z is BINARY (0/1 spikes), so
W_eff^T is split into three bf16 limbs (hi+mid+lo, 24 mantissa bits -- the
split reproduces the fp32 weights EXACTLY, verified on host) and the
product z*w_limb incurs no multiply rounding. Accumulation stays in fp32
PSUM, so the result matches the fp32 matmul up to accumulation order
(~1e-7), far inside the chaos tolerance of this spiking net (measured
end-to-end rel err 5.3e-3 vs the 2e-2 gate, same as the fp32 kernel). 48
bf16 MMs (2 psum banks x 3 limbs x 8 k-tiles, 512-wide moving) replace 16
fp32 MMs: ~10.2us vs ~14-18us of PE stream per step.

Structure per rep (one For_i over reps, so program size is nrep-independent;
the 99 steps are fully unrolled so precompute interleaves at instruction
granularity):
 1. I-precompute (fp32; X is not binary so bf16 splitting does not pay):
    I[h', tb] = W_in^T.T @ X^T into an h'-major DRAM ring, 7 chunks of <=512
    columns on psum banks 3..6 (two 4-h-tile passes per chunk). Chunk 0 is
    emitted up front; the rest is paced as 3-MM filler blocks (2/step then
    1/step, meeting each chunk's step-16c deadline) so the PE fills its
    tail-idle gaps with precompute instead of stalling.
 2. Scan, per step: 48 bf16 MMs (z-tiles stationary) into banks 0-1, with
    each bank's psum->SBUF copy overlapping the other bank's MMs; 8 PE
    transposes put the rec result into T-layout on bank 2; the v/z updates
    are split per 4-k-tile half so half 0's DVE ops hide under transposes
    4..7 and next-step k<4 matmuls unblock early; z is thresholded straight
    to bf16 (exact 0/1) as the next step's stationary. u stays in SBUF; the
    last 10 steps snapshot u (no u ring). I_t is double-buffered and DMA'd
    a step ahead. Step 0 skips the matmuls entirely (z=0 -> v1 = I_0).
 3. Epilogue: vo = W_out @ usnap as one 8-matmul fp32 chain, 320 free cols.

Host: X pre-transposed to [D, T*BL] per core; W_eff^T limb-split to bf16;
softmax+mean over the last 10 steps on host (0.005% of FLOPs).
"""

import sys

sys.path.insert(0, "/opt/trn_rl_repo")

import numpy as np
from contextlib import ExitStack

from concourse import bacc, bass, mybir, tile
from concourse.bass import ds, ts
from concourse.bass_utils import run_bass_kernel_spmd

F32 = mybir.dt.float32
BF16 = mybir.dt.bfloat16
A = mybir.AluOpType

B, T, D, H, O = 256, 100, 700, 1024, 20
NCORES = 8
BL = B // NCORES  # 32
KT = H // 128  # 8
DTILES = 6
DLAST = D - 5 * 128  # 60
NSTEPS = T - 1  # 99
NTAIL = 10
XCOLS = NSTEPS * BL  # 3168

ALPHA = float(np.float32(np.exp(-1.0 / 20.0)))
KAPPA = float(np.float32(np.exp(-1.0 / 20.0)))
THR = 1.0

WI_OFF = 0
WI_LEN = DTILES * H
WO_OFF = WI_OFF + WI_LEN
WO_LEN = KT * O
ID_OFF = WO_OFF + WO_LEN
ID_LEN = 32  # 32x32 identity for PE transposes (rows 0:32)
XT_OFF = ID_OFF + ID_LEN
XT_LEN = DTILES * XCOLS
BLOB_COLS = XT_OFF + XT_LEN  # 25344

# I-precompute chunks: 6x512 + 96 = 3168 columns
CHUNKS = [(i * 512, 512) for i in range(6)] + [(3072, 96)]


def _build(nsteps=NSTEPS, nrep=1):
    nc = bacc.Bacc(None)
    blob_d = nc.declare_dram_parameter("blob", [128, BLOB_COLS], F32, isOutput=False)
    wsp_d = nc.declare_dram_parameter("wsplit", [128, 3 * KT * H], BF16, isOutput=False)
    vo_d = nc.declare_dram_parameter("vo10", [O, NTAIL * BL], F32, isOutput=True)

    with ExitStack() as ctx:
        tc = ctx.enter_context(tile.TileContext(nc))
        const = ctx.enter_context(tc.tile_pool(name="const", bufs=1))
        pp = ctx.enter_context(tc.tile_pool(name="pp", bufs=1, space="PSUM"))
        dram = ctx.enter_context(tc.tile_pool(name="dram", bufs=1, space="DRAM"))

        blob_sb = const.tile([128, BLOB_COLS], F32, name="blob_sb")
        xt_sb = blob_sb[:, XT_OFF : XT_OFF + XT_LEN].rearrange(
            "p (a c) -> p a c", a=DTILES
        )
        wi_sb = blob_sb[:, WI_OFF : WI_OFF + WI_LEN].rearrange(
            "p (a c) -> p a c", a=DTILES
        )
        wo_sb = blob_sb[:, WO_OFF : WO_OFF + WO_LEN].rearrange(
            "p (a c) -> p a c", a=KT
        )
        ident = blob_sb[0:32, ID_OFF : ID_OFF + 32]
        # bf16 limb-split of W_eff^T: [128, limb, k-tile, h']
        wsp_sb = const.tile([128, 3, KT, H], BF16, name="wsp_sb")
        # T-layout state [128(neuron sub-tile p'), kt, b]; h' = kt*128 + p'
        state = const.tile([128, 2, KT, BL], F32, name="state")
        vT = state[:, 0]
        uT = state[:, 1]
        zbT = const.tile([128, KT, BL], BF16, name="zbT")
        iT2 = const.tile([128, 2, KT, BL], F32, name="iT2")  # double-buffered
        r_sb = const.tile([32, H], F32, name="r_sb")  # rec result, b-layout
        ibuf = const.tile([128, 4, 512], F32, name="ibuf")
        usnapT = const.tile([128, KT, NTAIL, BL], F32, name="usnapT")
        vo_sb = const.tile([O, NTAIL * BL], F32, name="vo_sb")
        ps = pp.tile([128, KT, 512], F32, name="ps")

        # h'-major I ring: addr = (kt*128+p')*cols + col
        iring = dram.tile([128, KT, XCOLS], F32)

        nc.sync.dma_start(blob_sb[:], blob_d[:])
        nc.sync.dma_start(
            wsp_sb[:].rearrange("p a b c -> p (a b c)"), wsp_d[:]
        )

        def precompute_gen():
            """fp32 I-precompute on psum banks 3..6, yielded in 6-MM blocks
            so the scan can interleave it into PE idle gaps."""
            for c0, cw in CHUNKS:
                for half in range(2):
                    for hh in range(4):
                        h = half * 4 + hh
                        for dk in range(DTILES):
                            w_ = 128 if dk < 5 else DLAST
                            nc.tensor.matmul(
                                ps[:, 3 + hh, 0:cw],
                                wi_sb[0:w_, dk, h * 128 : (h + 1) * 128],
                                xt_sb[0:w_, dk, c0 : c0 + cw],
                                start=(dk == 0),
                                stop=(dk == DTILES - 1),
                            )
                            if dk == 2:  # 3-MM blocks match idle-gap size
                                yield
                        yield
                    nc.vector.tensor_copy(
                        ibuf[:, :, 0:cw], ps[:, 3:7, 0:cw]
                    )
                    nc.gpsimd.dma_start(
                        iring[:, half * 4 : (half + 1) * 4, c0 : c0 + cw],
                        ibuf[:, :, 0:cw],
                    )
                    yield

        def scan_step(t):
            iT = iT2[:, t % 2]
            nc.sync.dma_start(iT, iring[:, :, t * BL : (t + 1) * BL])
            if t == 0:
                # z,v,u are all zero: v1 = I_0, z1 = v1 > thr, u1 = z1
                nc.vector.tensor_copy(vT[:], iT)
                nc.vector.tensor_scalar(zbT[:], vT[:], THR, None, A.is_gt)
                nc.vector.scalar_tensor_tensor(
                    uT[:], uT[:], KAPPA, zbT[:], A.mult, A.add
                )
                return
            # rec matmul: 2 psum banks x 3 bf16 limbs x 8 k-tiles; bank 0's
            # psum->SBUF copy overlaps bank 1's matmuls, bank 1's copy hides
            # under transposes k=0..3
            for c in range(2):
                for s in range(3):
                    for k in range(KT):
                        nc.tensor.matmul(
                            ps[0:32, c, 0:512],
                            zbT[:, k, :],
                            wsp_sb[:, s, k, c * 512 : (c + 1) * 512],
                            start=(s == 0 and k == 0),
                            stop=(s == 2 and k == KT - 1),
                        )
                nc.vector.tensor_copy(
                    r_sb[:, c * 512 : (c + 1) * 512], ps[0:32, c, 0:512]
                )
            # transpose rec result into T-layout on bank 2; the v/z updates
            # are split per 4-k-tile half so half 0's DVE ops run while the
            # PE does transposes 4..7, and the next step's k<4 matmuls only
            # wait on zbT's first half
            for half in range(2):
                sl = slice(half * 4, half * 4 + 4)
                for k in range(half * 4, half * 4 + 4):
                    nc.tensor.transpose(
                        ps[:, 2, k * BL : (k + 1) * BL],
                        r_sb[:, k * 128 : (k + 1) * 128],
                        ident,
                    )
                nc.vector.scalar_tensor_tensor(
                    vT[:, sl], vT[:, sl], ALPHA,
                    ps[:, 2, half * 4 * BL : (half * 4 + 4) * BL].rearrange(
                        "p (a c) -> p a c", a=4
                    ),
                    A.mult, A.add,
                )
                nc.vector.scalar_tensor_tensor(
                    vT[:, sl], vT[:, sl], 1.0, iT[:, sl], A.mult, A.add
                )
                nc.vector.tensor_scalar(
                    zbT[:, sl], vT[:, sl], THR, None, A.is_gt
                )
            nc.vector.scalar_tensor_tensor(
                uT[:], uT[:], KAPPA, zbT[:], A.mult, A.add
            )
            j = t - (nsteps - NTAIL)
            if j >= 0:
                nc.vector.tensor_copy(usnapT[:, :, j, :], uT[:])

        with tc.For_i(0, nrep) as rep:
            nc.any.memzero(state[:])
            nc.any.memzero(zbT[:])

            gen = precompute_gen()
            for _ in range(18):  # chunk 0 up front (steps 0-15 covered)
                next(gen)
            # remaining 108 yields (18 per chunk): 2/step through step 26
            # puts chunk c done by step ~9c-1 (deadline 16c), then 1/step
            # finishes chunk 4 by 44, 5 by 62, 6 by 80 (deadlines 64/80/96)
            done = False
            for t in range(nsteps):
                scan_step(t)
                npull = 2 if t < 27 else 1
                for _ in range(npull):
                    if not done:
                        try:
                            next(gen)
                        except StopIteration:
                            done = True

            # ---- epilogue: vo[20, 320] = W_out @ usnap in one chain ----
            for k in range(KT):
                nc.tensor.matmul(
                    ps[0:O, 0, 0 : NTAIL * BL],
                    wo_sb[:, k, :],
                    usnapT[:, k, :, :],
                    start=(k == 0),
                    stop=(k == KT - 1),
                )
            nc.vector.tensor_copy(vo_sb[:], ps[0:O, 0, 0 : NTAIL * BL])
            nc.gpsimd.dma_start(vo_d[:], vo_sb[:])

    nc.compile()
    return nc


_PROGRAM = None


def _get_program():
    global _PROGRAM
    if _PROGRAM is None:
        _PROGRAM = _build()
    return _PROGRAM


def _host_prep(W_in, W_rec, W_out):
    """fp32 blob part (W_in^T, W_out^T, identity) + bf16 limb-split of
    W_eff^T ([128, 3*KT*H] bf16, h-sub-tile partition-major)."""
    import ml_dtypes

    bf = ml_dtypes.bfloat16
    eye = np.eye(H, dtype=np.float32)
    WrT = (W_rec * (1.0 - eye) - np.float32(THR) * eye).T.astype(np.float32)
    hi = WrT.astype(bf).astype(np.float32)
    mid = (WrT - hi).astype(bf).astype(np.float32)
    lo = (WrT - hi - mid).astype(bf)
    limbs = np.stack([hi.astype(bf), mid.astype(bf), lo], axis=0)  # [3, H, H]
    # -> [128, 3, KT, H]: partition p' = h % 128, k-tile = h // 128
    wsplit = np.ascontiguousarray(
        limbs.reshape(3, KT, 128, H).transpose(2, 0, 1, 3).reshape(128, -1)
    )

    WiT = np.zeros((DTILES * 128, H), np.float32)
    WiT[:D] = W_in.T.astype(np.float32)
    WoT = W_out.T.astype(np.float32)
    idpart = np.zeros((128, 32), np.float32)
    idpart[:32] = np.eye(32, dtype=np.float32)
    wpart = np.concatenate(
        [
            WiT.reshape(DTILES, 128, H).transpose(1, 0, 2).reshape(128, -1),
            WoT.reshape(KT, 128, O).transpose(1, 0, 2).reshape(128, -1),
            idpart,
        ],
        axis=1,
    )
    return np.ascontiguousarray(wpart), wsplit


def _make_in_maps(X, W_in, W_rec, W_out):
    X = np.asarray(X, np.float32)
    wpart, wsplit = _host_prep(
        np.asarray(W_in, np.float32), np.asarray(W_rec, np.float32),
        np.asarray(W_out, np.float32),
    )
    in_maps = []
    for c in range(NCORES):
        Xc = X[c * BL : (c + 1) * BL]
        XTc = np.zeros((DTILES * 128, XCOLS), np.float32)
        XTc[:D] = Xc[:, :NSTEPS, :].transpose(2, 1, 0).reshape(D, XCOLS)
        blob = np.concatenate(
            [wpart,
             XTc.reshape(DTILES, 128, XCOLS).transpose(1, 0, 2).reshape(128, -1)],
            axis=1,
        )
        in_maps.append(
            {"blob": np.ascontiguousarray(blob), "wsplit": wsplit}
        )
    return in_maps


def kernel(X, W_in, W_rec, W_out):
    nc = _get_program()
    in_maps = _make_in_maps(X, W_in, W_rec, W_out)
    res = run_bass_kernel_spmd(nc, in_maps, list(range(NCORES)))
    vo = np.stack([r["vo10"] for r in res.results])
    vo = vo.reshape(NCORES, O, NTAIL, BL).transpose(2, 0, 3, 1).reshape(NTAIL, B, O)
    m = vo.max(axis=2, keepdims=True)
    e = np.exp(vo - m)
    yo = e / e.sum(axis=2, keepdims=True)
    return yo.mean(axis=0).astype(np.float32)
